# revision 19
# baseline (speedup 1.0000x reference)
"""Causal self-attention (B=4, T=2048, C=768, H=12, RoPE) on 8 TRN2 NeuronCores.

Sharding: core c -> (batch b = c//2, head-group g = c%2 of 6 heads).
Each core computes, for its batch element and its 6 heads, the qkv
projections, RoPE, causal attention, and the partial output projection
attn_out @ W_proj[rows of its heads].  Host sums the two partial outputs
per batch and adds the (host-folded) bias terms:
    out[b] = part[2b] + part[2b+1] + b_proj + b_v @ W_proj.

v2 design (all matmul operands bf16, fp32 PSUM accumulation):
  - Inputs arrive in TWO packed DRAM tensors (wall1/wall2) laid out
    [128, cols] so a single DMA each fills SBUF; all weight/table/bias
    slices are views into them.
  - RoPE rotate-half is a DVE stream_shuffle: the head-dim columns of
    Wq/Wk (and cos/sin tables, biases) are permuted host-side so each
    (x1_i, x2_i) pair sits in the same 32-partition quadrant.
  - Scores computed transposed per head pair: S^T[k,q] blocks [128,1024]
    (two heads), exp on ACT -> bf16 pp, causal mask via affine_select on
    GPSIMD, then [V|1]-chain accumulation gives Y^T and softmax
    denominators in one PSUM tile per head.
  - Normalization: DVE reciprocal of the denominator row, GPSIMD
    partition_broadcast to 64 partitions, DVE multiply.  Head B's
    normalized block moves to partitions 64-127 via one SB->SB DMA.
  - Output projection in bf16, streamed out in 4 grouped DMAs.
"""
import sys
sys.path.insert(0, "/opt/trn_rl_repo")

import numpy as np
from ml_dtypes import bfloat16

ROPE_BASE = 10000.0
NCORES = 8

_CACHE = {}

# new head-dim position j holds old component _PERM[j]; pairs (x1_i, x2_i)
# are 16 apart so the rotate-half is a within-quadrant 32-shuffle.
_PERM = np.array([*range(0, 16), *range(32, 48), *range(16, 32), *range(48, 64)])
_SHUF_MASK = [*range(16, 32), *range(0, 16)]


def _rope_tables(T):
    """cos/sin tables [128, T] matching the permuted head-dim layout."""
    j = np.arange(64)
    fi = (j % 16) + 16 * (j // 32)          # frequency index per new row
    sign = np.where((j // 16) % 2 == 0, 1.0, -1.0)  # +sin for x1 rows
    inv_freq = 1.0 / (ROPE_BASE ** (np.arange(0, 64, 2, dtype=np.float64) / 64))
    t = np.arange(T, dtype=np.float64)
    ang = np.outer(inv_freq[fi], t)         # [64, T]
    cc = np.tile(np.cos(ang), (2, 1))       # [128, T]
    ss = np.tile(np.sin(ang) * sign[:, None], (2, 1))
    return cc.astype(np.float32), ss.astype(np.float32)


def build_nc(C, T, HPC, debug=False, loop_n=1):
    """Build the per-core Bass program. C: contraction dim, T: seq len,
    HPC: heads per core (even)."""
    import concourse.bass as bass
    import concourse.tile as tile
    from concourse import bacc, mybir

    F32 = mybir.dt.float32
    BF16 = mybir.dt.bfloat16
    Act = mybir.ActivationFunctionType

    KT = C // 128          # contraction k-tiles
    NP = HPC // 2          # head pairs
    TT = T // 128          # 128-row t-tiles (= key chunks)
    QC = T // 512          # query chunks of 512
    VC = 64 * HPC          # v columns
    PC = VC // 128         # projection contraction k-tiles (= NP)

    # wall1 column offsets (bf16); wqk stored as KT slabs of 2*VC columns
    OXT = 0
    OWQK = OXT + KT * T
    OCC = OWQK + KT * (2 * VC)
    OSS = OCC + T
    OBQ = OSS + T
    NC1 = OBQ + 16
    OWV = 0
    OWP = OWV + KT * VC
    NC2 = OWP + PC * C

    nc = bacc.Bacc("TRN2", target_bir_lowering=False, debug=False)

    wall1_d = nc.dram_tensor("wall1", [128, NC1], BF16, kind="ExternalInput")
    wall2_d = nc.dram_tensor("wall2", [128, NC2], BF16, kind="ExternalInput")
    out_d = nc.dram_tensor("out", [T, C], BF16, kind="ExternalOutput")
    if debug:
        dbg_qk = nc.dram_tensor("dbg_qk", [2 * NP, 128, T], BF16,
                                kind="ExternalOutput")
        dbg_vp = nc.dram_tensor("dbg_vp", [128, TT * HPC * 65], BF16,
                                kind="ExternalOutput")
        dbg_yt = nc.dram_tensor("dbg_yt", [NP, 128, T], BF16,
                                kind="ExternalOutput")
        dbg_ra = nc.dram_tensor("dbg_ra", [NP * (T // 512), 128, 512], F32,
                                kind="ExternalOutput")

    import contextlib

    @contextlib.contextmanager
    def _maybe_loop(tc):
        if loop_n > 1:
            with tc.For_i(0, loop_n, 1):
                yield
        else:
            yield

    with nc.allow_low_precision(reason="bf16 matmul pipeline"), \
         tile.TileContext(nc) as tc:
        with contextlib.ExitStack() as octx, _maybe_loop(tc), \
             contextlib.ExitStack() as ctx:
            # ---- long-lived pools -------------------------------------
            wallp = ctx.enter_context(tc.tile_pool(name="wall", bufs=1))
            vp_pool = ctx.enter_context(tc.tile_pool(name="vp", bufs=1))
            qk_pool = ctx.enter_context(tc.tile_pool(name="qks", bufs=2))
            yt_pool = ctx.enter_context(tc.tile_pool(name="yt", bufs=1))

            w1 = wallp.tile([128, NC1], BF16)
            w2 = wallp.tile([128, NC2], BF16)
            vp_sb = vp_pool.tile([128, TT, HPC, 65], BF16)
            yt_sb = yt_pool.tile([128, NP, T], BF16)

            nc.vector.tensor_copy(
                vp_sb[:, :, :, 64:65],
                nc.const_aps.tensor(1.0, (128, TT, HPC, 1)))

            nc.sync.dma_start(w1[:], wall1_d.ap())
            nc.sync.dma_start(w2[:], wall2_d.ap())

            bqk_f32 = wallp.tile([128, 16], mybir.dt.float32)
            ones_sb = wallp.tile([128, 64], mybir.dt.float32r)
            nc.vector.tensor_copy(ones_sb[64:128, :],
                                  nc.const_aps.tensor(1.0, (64, 64)))

            xt = lambda k: w1[:, OXT + k * T: OXT + (k + 1) * T]
            wqk = lambda k: w1[:, OWQK + k * 2 * VC: OWQK + (k + 1) * 2 * VC]
            ccv = w1[:, OCC:OCC + T]
            ssv = w1[:, OSS:OSS + T]
            bqk = bqk_f32[:]
            nc.vector.tensor_copy(bqk_f32, w1[:, OBQ:OBQ + 16])
            wv = lambda k: w2[:, OWV + k * VC: OWV + (k + 1) * VC]
            wpv = lambda k: w2[:, OWP + k * C: OWP + (k + 1) * C]

            # ---- per-pair QK+RoPE, V projection, attention ------------
            raw_pool = ctx.enter_context(tc.tile_pool(name="raw", bufs=2))
            swp_pool = ctx.enter_context(tc.tile_pool(name="swp", bufs=2))
            a1_pool = ctx.enter_context(tc.tile_pool(name="a1", bufs=2))

            with tc.tile_pool(name="mmps", bufs=2, space="PSUM") as mmps, \
                 tc.tile_pool(name="yps", bufs=2, space="PSUM") as yps, \
                 tc.tile_pool(name="pt", bufs=3) as pt_pool, \
                 tc.tile_pool(name="ra", bufs=2) as ra_pool, \
                 tc.tile_pool(name="tb", bufs=2) as tb_pool:
                for j in range(NP):
                    qt = qk_pool.tile([128, T], BF16, tag="qt", name=f"qt{j}")
                    kt = qk_pool.tile([128, T], BF16, tag="kt", name=f"kt{j}")
                    QKW = 1024 if T % 1024 == 0 else 512
                    for dst, m in ((qt, j), (kt, NP + j)):
                        for ch in range(T // QKW):
                            psum = mmps.tile([128, 1024], F32, tag="mm",
                                             name="qkpsum")
                            for half in range(QKW // 512):
                                hs = slice(half * 512, (half + 1) * 512)
                                xs = slice(ch * QKW + half * 512,
                                           ch * QKW + (half + 1) * 512)
                                for k in range(KT):
                                    nc.tensor.matmul(
                                        psum[:, hs],
                                        wqk(k)[:, m * 128:(m + 1) * 128],
                                        xt(k)[:, xs],
                                        start=(k == 0), stop=(k == KT - 1))
                            cs = slice(ch * QKW, (ch + 1) * QKW)
                            raw = raw_pool.tile([128, QKW], BF16)
                            nc.vector.tensor_scalar_add(
                                raw, psum[:, 0:QKW], bqk[:, m:m + 1])
                            swp = swp_pool.tile([128, QKW], BF16)
                            nc.vector.stream_shuffle(swp, raw, _SHUF_MASK)
                            a1 = a1_pool.tile([128, QKW], BF16)
                            nc.vector.tensor_mul(a1, raw, ccv[:, cs])
                            nc.gpsimd.tensor_mul(swp, swp, ssv[:, cs])
                            nc.vector.tensor_add(dst[:, cs], a1, swp)

                    if j == 0:
                        # V projection after pair-0 QK: overlaps attention
                        for tt in range(TT):
                            vpsum = mmps.tile([128, 1024], F32, tag="mm",
                                              name="vpsum")
                            for k in range(KT):
                                nc.tensor.matmul(
                                    vpsum[:, 0:VC],
                                    xt(k)[:, tt * 128:(tt + 1) * 128],
                                    wv(k),
                                    start=(k == 0), stop=(k == KT - 1))
                            nc.vector.tensor_copy(
                                vp_sb[:, tt, :, 0:64],
                                vpsum[:, 0:VC].rearrange("p (h d) -> p h d",
                                                         h=HPC))

                    # attention for pair j
                    for qc in range(QC):
                        nkc = 4 * (qc + 1)
                        ya = yps.tile([65, 512], F32, tag="ya", name="ya")
                        yb = yps.tile([65, 512], F32, tag="yb", name="yb")
                        for kc in range(nkc):
                            qs = slice(qc * 512, (qc + 1) * 512)
                            ks = slice(kc * 128, (kc + 1) * 128)
                            spair = mmps.tile([128, 1024], F32, tag="mm",
                                              name="spair")
                            nc.tensor.matmul(spair[:, 0:512], kt[0:64, ks],
                                             qt[0:64, qs], start=True, stop=True)
                            nc.tensor.matmul(spair[:, 512:1024], kt[64:128, ks],
                                             qt[64:128, qs], start=True, stop=True)
                            pp = pt_pool.tile([128, 1024], BF16, tag="pp")
                            nc.scalar.activation(pp, spair, Act.Exp, scale=0.125)
                            if kc >= 4 * qc:  # diagonal: mask k > q -> 0
                                nc.gpsimd.affine_select(
                                    out=pp, in_=pp,
                                    compare_op=mybir.AluOpType.is_ge,
                                    fill=0.0,
                                    base=qc * 512 - kc * 128,
                                    channel_multiplier=-1,
                                    pattern=[[0, 2], [1, 512]])
                            nc.tensor.matmul(ya, vp_sb[:, kc, 2 * j, :],
                                             pp[:, 0:512],
                                             start=(kc == 0), stop=(kc == nkc - 1))
                            nc.tensor.matmul(yb, vp_sb[:, kc, 2 * j + 1, :],
                                             pp[:, 512:1024],
                                             start=(kc == 0), stop=(kc == nkc - 1))
                        qs = slice(qc * 512, (qc + 1) * 512)
                        ra = ra_pool.tile([128, 512], mybir.dt.float32r,
                                          tag="ra")
                        rb = ra_pool.tile([128, 512], mybir.dt.float32r,
                                          tag="rb")
                        nc.vector.reciprocal(ra[64:65, 0:512], ya[64:65, :])
                        nc.vector.reciprocal(rb[64:65, 0:512], yb[64:65, :])
                        bc = mmps.tile([128, 1024], F32, tag="mm", name="bc")
                        nc.tensor.matmul(bc[0:64, 0:512], ones_sb[64:65, :],
                                         ra[64:65, 0:512],
                                         start=True, stop=True)
                        nc.tensor.matmul(bc[0:64, 512:1024], ones_sb[64:65, :],
                                         rb[64:65, 0:512],
                                         start=True, stop=True)
                        bcs = tb_pool.tile([64, 1024], mybir.dt.float32r,
                                           tag="bcs")
                        nc.vector.tensor_copy(bcs, bc[0:64, :])
                        nc.vector.tensor_mul(yt_sb[0:64, j, qs], ya[0:64, :],
                                             bcs[:, 0:512])
                        tb = tb_pool.tile([64, 512], BF16)
                        nc.vector.tensor_mul(tb, yb[0:64, :], bcs[:, 512:1024])
                        nc.sync.dma_start(yt_sb[64:128, j, qs], tb)
                        if debug:
                            nc.sync.dma_start(dbg_ra.ap()[j * QC + qc, 64:65],
                                              ra[64:65, :].bitcast(F32))

                    if debug:
                        nc.sync.dma_start(dbg_qk.ap()[j], qt[:])
                        nc.sync.dma_start(dbg_qk.ap()[NP + j], kt[:])

                if debug:
                    nc.sync.dma_start(
                        dbg_vp.ap(),
                        vp_sb[:].rearrange("p a b c -> p (a b c)"))
                    for j in range(NP):
                        nc.sync.dma_start(dbg_yt.ap()[j], yt_sb[:, j, :])

            # ---- output projection ------------------------------------
            with tc.tile_pool(name="osb", bufs=2) as osb_pool, \
                 tc.tile_pool(name="pps", bufs=3, space="PSUM") as pps:
                ccw = 384
                for tg in range(TT // 4):
                    osb = osb_pool.tile([128, 4, C], BF16)
                    for a in range(4):
                        tt = tg * 4 + a
                        for c2 in range(C // ccw):
                            cs = slice(c2 * ccw, (c2 + 1) * ccw)
                            psum = pps.tile([128, ccw], F32)
                            for k in range(PC):
                                nc.tensor.matmul(
                                    psum, yt_sb[:, k, tt * 128:(tt + 1) * 128],
                                    wpv(k)[:, cs],
                                    start=(k == 0), stop=(k == PC - 1))
                            if c2 == 0:
                                nc.vector.tensor_copy(osb[:, a, cs], psum)
                            else:
                                nc.scalar.copy(osb[:, a, cs], psum)
                    nc.sync.dma_start(
                        out_d.ap()[tg * 512:(tg + 1) * 512, :]
                             .rearrange("(a p) c -> p a c", p=128),
                        osb[:])

    nc.compile()
    return nc


class _Runner:
    """Cached-jit SPMD runner (mirrors bass2jax.run_bass_via_pjrt, reusable)."""

    def __init__(self, nc, n_cores):
        import jax
        from jax.sharding import Mesh, PartitionSpec
        from jax.experimental.shard_map import shard_map
        import concourse.mybir as mybir
        from concourse import bass2jax

        bass2jax.install_neuronx_cc_hook()
        self.n_cores = n_cores
        part_name = (nc.partition_id_tensor.name
                     if nc.partition_id_tensor is not None else None)
        in_names, out_names, out_avals, zero_outs = [], [], [], []
        for alloc in nc.m.functions[0].allocations:
            if not isinstance(alloc, mybir.MemoryLocationSet):
                continue
            name = alloc.memorylocations[0].name
            if alloc.kind == "ExternalInput":
                if name != part_name:
                    in_names.append(name)
            elif alloc.kind == "ExternalOutput":
                out_names.append(name)
                shape = tuple(alloc.tensor_shape)
                dtype = mybir.dt.np(alloc.dtype)
                out_avals.append(jax.core.ShapedArray(shape, dtype))
                zero_outs.append(np.zeros(shape, dtype))
        self.in_names, self.out_names = in_names, out_names
        self.out_avals, self.zero_outs = out_avals, zero_outs
        all_names = in_names + out_names
        if part_name is not None:
            all_names = all_names + [part_name]

        def _body(*args):
            operands = list(args)
            if part_name is not None:
                operands.append(bass2jax.partition_id_tensor())
            return tuple(bass2jax._bass_exec_p.bind(
                *operands,
                out_avals=tuple(out_avals),
                in_names=tuple(all_names),
                out_names=tuple(out_names),
                lowering_input_output_aliases=(),
                sim_require_finite=True,
                sim_require_nnan=True,
                nc=nc,
            ))

        devices = jax.devices()[:n_cores]
        mesh = Mesh(np.asarray(devices), ("core",))
        nin = len(in_names) + len(out_names)
        self._fn = jax.jit(
            shard_map(_body, mesh=mesh,
                      in_specs=(PartitionSpec("core"),) * nin,
                      out_specs=(PartitionSpec("core"),) * len(out_names),
                      check_rep=False),
            keep_unused=True)

    def run(self, in_maps):
        args = [np.concatenate([np.asarray(m[name]) for m in in_maps], axis=0)
                for name in self.in_names]
        args += [np.zeros((self.n_cores * z.shape[0], *z.shape[1:]), z.dtype)
                 for z in self.zero_outs]
        outs = self._fn(*args)
        res = []
        for c in range(self.n_cores):
            d = {}
            for i, name in enumerate(self.out_names):
                per = np.asarray(outs[i]).reshape(
                    self.n_cores, *self.out_avals[i].shape)
                d[name] = per[c]
            res.append(d)
        return res


def _run(nc, in_maps):
    key = ("runner", id(nc))
    if key not in _CACHE:
        _CACHE[key] = _Runner(nc, len(in_maps))
    import types
    return types.SimpleNamespace(results=_CACHE[key].run(in_maps))


def build_in_maps(x, W, b, Wp, T, C):
    """Host-side packing of the per-core wall1/wall2 tensors (bf16)."""
    KT = C // 128
    cc, ss = _rope_tables(T)
    heads01 = np.arange(128) // 64
    jj = np.arange(128) % 64
    in_maps = []
    for c in range(NCORES):
        bb, g = divmod(c, 2)
        s = 384 * g
        wqk = np.empty((C, 768), np.float32)
        bqk = np.zeros((128, 16), np.float32)
        for m in range(3):
            for h01 in range(2):
                h = 2 * m + h01
                cols = slice(m * 128 + h01 * 64, m * 128 + h01 * 64 + 64)
                wqk[:, cols] = W[:, s + h * 64 + _PERM]
                wqk[:, 384 + m * 128 + h01 * 64:
                       384 + m * 128 + h01 * 64 + 64] = \
                    W[:, 768 + s + h * 64 + _PERM]
            bqk[:, m] = b[s + (2 * m + heads01) * 64 + _PERM[jj]]
            bqk[:, 3 + m] = b[768 + s + (2 * m + heads01) * 64 + _PERM[jj]]
        xt = np.ascontiguousarray(x[bb].T)                      # [C, T]
        wvh = W[:, 1536 + s:1536 + s + 384]                     # [C, 384]
        wph = Wp[s:s + 384, :]                                  # [384, C]
        wall1 = np.concatenate([
            xt.reshape(KT, 128, T).transpose(1, 0, 2).reshape(128, KT * T),
            wqk.reshape(KT, 128, 768).transpose(1, 0, 2).reshape(128, -1),
            cc, ss, bqk], axis=1).astype(bfloat16)
        wall2 = np.concatenate([
            wvh.reshape(KT, 128, 384).transpose(1, 0, 2).reshape(128, -1),
            wph.reshape(3, 128, C).transpose(1, 0, 2).reshape(128, -1),
        ], axis=1).astype(bfloat16)
        in_maps.append({"wall1": np.ascontiguousarray(wall1),
                        "wall2": np.ascontiguousarray(wall2)})
    return in_maps


def kernel(**inputs):
    x = np.ascontiguousarray(np.asarray(inputs["x"], dtype=np.float32))
    W = np.asarray(inputs["W_attn"], dtype=np.float32)
    b = np.asarray(inputs["b_attn"], dtype=np.float32)
    Wp = np.asarray(inputs["W_proj"], dtype=np.float32)
    bp = np.asarray(inputs["b_proj"], dtype=np.float32)
    B, T, C = x.shape

    if "nc" not in _CACHE:
        _CACHE["nc"] = build_nc(C, T, 6)
        _CACHE["build_args"] = (C, T, 6)
    nc = _CACHE["nc"]

    in_maps = build_in_maps(x, W, b, Wp, T, C)
    _CACHE["in_maps"] = in_maps
    res = _run(nc, in_maps).results
    extra = (bp + b[1536:2304] @ Wp).astype(np.float32)  # [C]
    out = np.empty((B, T, C), dtype=np.float32)
    for bb in range(B):
        out[bb] = (res[2 * bb]["out"].astype(np.float32)
                   + res[2 * bb + 1]["out"].astype(np.float32) + extra)
    return out


# revision 33
# speedup vs baseline: 1.1390x; 1.1390x over previous
"""Causal self-attention (B=4, T=2048, C=768, H=12, RoPE) on 8 TRN2 NeuronCores.

Sharding: core c -> (batch b = c//2, head-group g = c%2 of 6 heads).
Each core computes, for its batch element and its 6 heads, the qkv
projections, RoPE, causal attention, and the partial output projection
attn_out @ W_proj[rows of its heads].  Host sums the two partial outputs
per batch and adds the (host-folded) bias terms:
    out[b] = part[2b] + part[2b+1] + b_proj + b_v @ W_proj.

v2 design (all matmul operands bf16, fp32 PSUM accumulation):
  - Inputs arrive in TWO packed DRAM tensors (wall1/wall2) laid out
    [128, cols] so a single DMA each fills SBUF; all weight/table/bias
    slices are views into them.
  - RoPE rotate-half is a DVE stream_shuffle: the head-dim columns of
    Wq/Wk (and cos/sin tables, biases) are permuted host-side so each
    (x1_i, x2_i) pair sits in the same 32-partition quadrant.
  - Scores computed transposed per head pair: S^T[k,q] blocks [128,1024]
    (two heads), exp on ACT -> bf16 pp, causal mask via affine_select on
    GPSIMD, then [V|1]-chain accumulation gives Y^T and softmax
    denominators in one PSUM tile per head.
  - Normalization: DVE reciprocal of the denominator row, GPSIMD
    partition_broadcast to 64 partitions, DVE multiply.  Head B's
    normalized block moves to partitions 64-127 via one SB->SB DMA.
  - Output projection in bf16, streamed out in 4 grouped DMAs.
"""
import sys
sys.path.insert(0, "/opt/trn_rl_repo")

import numpy as np
from ml_dtypes import bfloat16

ROPE_BASE = 10000.0
NCORES = 8

_CACHE = {}

# new head-dim position j holds old component _PERM[j]; pairs (x1_i, x2_i)
# are 16 apart so the rotate-half is a within-quadrant 32-shuffle.
_PERM = np.array([*range(0, 16), *range(32, 48), *range(16, 32), *range(48, 64)])
_SHUF_MASK = [*range(16, 32), *range(0, 16)]


def _rope_tables(T):
    """cos/sin tables [128, T] matching the permuted head-dim layout."""
    j = np.arange(64)
    fi = (j % 16) + 16 * (j // 32)          # frequency index per new row
    sign = np.where((j // 16) % 2 == 0, 1.0, -1.0)  # +sin for x1 rows
    inv_freq = 1.0 / (ROPE_BASE ** (np.arange(0, 64, 2, dtype=np.float64) / 64))
    t = np.arange(T, dtype=np.float64)
    ang = np.outer(inv_freq[fi], t)         # [64, T]
    cc = np.tile(np.cos(ang), (2, 1))       # [128, T]
    ss = np.tile(np.sin(ang) * sign[:, None], (2, 1))
    return cc.astype(np.float32), ss.astype(np.float32)


def build_nc(C, T, HPC, debug=False, loop_n=1, probe=()):
    """Build the per-core Bass program. C: contraction dim, T: seq len,
    HPC: heads per core (even).

    probe: timing-probe variants (break correctness, keep structure):
      "noexp"  - AV matmuls read a constant pp (exp still runs, chain cut)
      "nomask" - skip affine_select
      "nonorm" - yt written by plain copy (no recip/broadcast/tb DMA)
      "noattn" - skip score/exp/AV entirely (yt memset once)
      "noproj" - skip output projection (tiny out DMA instead)
    """
    probe = set(probe)
    import concourse.bass as bass
    import concourse.tile as tile
    from concourse import bacc, mybir

    F32 = mybir.dt.float32
    BF16 = mybir.dt.bfloat16
    Act = mybir.ActivationFunctionType

    KT = C // 128          # contraction k-tiles
    NP = HPC // 2          # head pairs
    TT = T // 128          # 128-row t-tiles (= key chunks)
    QC = T // 512          # query chunks of 512
    VC = 64 * HPC          # v columns
    PC = VC // 128         # projection contraction k-tiles (= NP)

    # wall1 column offsets (bf16); wqk stored as KT slabs of 2*VC columns
    OXT = 0
    OWQK = OXT + KT * T
    OCC = OWQK + KT * (2 * VC)
    OSS = OCC + T
    OBQ = OSS + T
    NC1 = OBQ + 16
    OWV = 0
    OWP = OWV + KT * VC
    NC2 = OWP + PC * C

    nc = bacc.Bacc("TRN2", target_bir_lowering=False, debug=False)

    wall1_d = nc.dram_tensor("wall1", [128, NC1], BF16, kind="ExternalInput")
    wall2_d = nc.dram_tensor("wall2", [128, NC2], BF16, kind="ExternalInput")
    out_d = nc.dram_tensor("out", [T, C], BF16, kind="ExternalOutput")
    if debug:
        dbg_qk = nc.dram_tensor("dbg_qk", [2 * NP, 128, T], BF16,
                                kind="ExternalOutput")
        dbg_vp = nc.dram_tensor("dbg_vp", [128, TT * HPC * 65], BF16,
                                kind="ExternalOutput")
        dbg_yt = nc.dram_tensor("dbg_yt", [NP, 128, T], BF16,
                                kind="ExternalOutput")
        dbg_ra = nc.dram_tensor("dbg_ra", [NP * (T // 512), 128, 512], F32,
                                kind="ExternalOutput")

    import contextlib

    @contextlib.contextmanager
    def _maybe_loop(tc):
        if loop_n > 1:
            with tc.For_i(0, loop_n, 1):
                yield
        else:
            yield

    with nc.allow_low_precision(reason="bf16 matmul pipeline"), \
         tile.TileContext(nc) as tc:
        with contextlib.ExitStack() as octx, _maybe_loop(tc), \
             contextlib.ExitStack() as ctx:
            # ---- long-lived pools -------------------------------------
            wallp = ctx.enter_context(tc.tile_pool(name="wall", bufs=1))
            vp_pool = ctx.enter_context(tc.tile_pool(name="vp", bufs=1))
            qk_pool = ctx.enter_context(tc.tile_pool(name="qks", bufs=2))
            yt_pool = ctx.enter_context(tc.tile_pool(name="yt", bufs=1))

            w1 = wallp.tile([128, NC1], BF16)
            w2 = wallp.tile([128, NC2], BF16)
            vp_sb = vp_pool.tile([128, TT, HPC, 65], BF16)
            yt_sb = yt_pool.tile([128, NP, T], BF16)

            nc.vector.tensor_copy(
                vp_sb[:, :, :, 64:65],
                nc.const_aps.tensor(1.0, (128, TT, HPC, 1)))

            nc.sync.dma_start(w1[:], wall1_d.ap())
            nc.sync.dma_start(w2[:], wall2_d.ap())

            bqk_f32 = wallp.tile([128, 16], mybir.dt.float32)
            ones_sb = wallp.tile([128, 64], mybir.dt.float32r)
            nc.vector.tensor_copy(ones_sb[64:128, :],
                                  nc.const_aps.tensor(1.0, (64, 64)))

            xt = lambda k: w1[:, OXT + k * T: OXT + (k + 1) * T]
            wqk = lambda k: w1[:, OWQK + k * 2 * VC: OWQK + (k + 1) * 2 * VC]
            ccv = w1[:, OCC:OCC + T]
            ssv = w1[:, OSS:OSS + T]
            bqk = bqk_f32[:]
            nc.vector.tensor_copy(bqk_f32, w1[:, OBQ:OBQ + 16])
            wv = lambda k: w2[:, OWV + k * VC: OWV + (k + 1) * VC]
            wpv = lambda k: w2[:, OWP + k * C: OWP + (k + 1) * C]

            # ---- per-pair QK+RoPE, V projection, attention ------------
            raw_pool = ctx.enter_context(tc.tile_pool(name="raw", bufs=2))
            swp_pool = ctx.enter_context(tc.tile_pool(name="swp", bufs=2))
            a1_pool = ctx.enter_context(tc.tile_pool(name="a1", bufs=2))

            with tc.tile_pool(name="mmps", bufs=3, space="PSUM") as mmps, \
                 tc.tile_pool(name="yps", bufs=1, space="PSUM") as yps, \
                 tc.tile_pool(name="pt", bufs=4) as pt_pool, \
                 tc.tile_pool(name="ra", bufs=2) as ra_pool, \
                 tc.tile_pool(name="yu", bufs=2) as yu_pool, \
                 tc.tile_pool(name="tb", bufs=2) as tb_pool:
                if "noexp" in probe:
                    ppc = pt_pool.tile([128, 1024], BF16, tag="ppc")
                    nc.vector.tensor_copy(
                        ppc, nc.const_aps.tensor(1.0, (128, 1024)))
                if "noattn" in probe or "nonorm" in probe:
                    nc.vector.tensor_copy(
                        yt_sb[:], nc.const_aps.tensor(1.0, (128, NP, T)))
                pending_bc = []
                for j in range(NP):
                    qt = qk_pool.tile([128, T], BF16, tag="qt", name=f"qt{j}")
                    kt = qk_pool.tile([128, T], BF16, tag="kt", name=f"kt{j}")
                    QKW = 1024 if T % 1024 == 0 else 512
                    for dst, m in ((qt, j), (kt, NP + j)):
                        for ch in range(T // QKW):
                            psum = mmps.tile([128, 1024], F32, tag="mm",
                                             name="qkpsum")
                            for half in range(QKW // 512):
                                hs = slice(half * 512, (half + 1) * 512)
                                xs = slice(ch * QKW + half * 512,
                                           ch * QKW + (half + 1) * 512)
                                for k in range(KT):
                                    nc.tensor.matmul(
                                        psum[:, hs],
                                        wqk(k)[:, m * 128:(m + 1) * 128],
                                        xt(k)[:, xs],
                                        start=(k == 0), stop=(k == KT - 1))
                            cs = slice(ch * QKW, (ch + 1) * QKW)
                            raw = raw_pool.tile([128, QKW], BF16)
                            nc.vector.tensor_scalar_add(
                                raw, psum[:, 0:QKW], bqk[:, m:m + 1])
                            swp = swp_pool.tile([128, QKW], BF16)
                            nc.vector.stream_shuffle(swp, raw, _SHUF_MASK)
                            a1 = a1_pool.tile([128, QKW], BF16)
                            nc.vector.tensor_mul(a1, raw, ccv[:, cs])
                            nc.gpsimd.tensor_mul(swp, swp, ssv[:, cs])
                            nc.vector.tensor_add(dst[:, cs], a1, swp)

                    if j == 0:
                        # V projection after pair-0 QK: overlaps attention
                        for tt in range(TT):
                            vpsum = mmps.tile([128, 1024], F32, tag="mm",
                                              name="vpsum")
                            for k in range(KT):
                                nc.tensor.matmul(
                                    vpsum[:, 0:VC],
                                    xt(k)[:, tt * 128:(tt + 1) * 128],
                                    wv(k),
                                    start=(k == 0), stop=(k == KT - 1))
                            nc.vector.tensor_copy(
                                vp_sb[:, tt, :, 0:64],
                                vpsum[:, 0:VC].rearrange("p (h d) -> p h d",
                                                         h=HPC))

                    # attention for pair j.  AV matmuls trail the score/exp
                    # chain by AVLAG blocks (software pipelining) so the
                    # in-order PE queue never stalls on an exp in flight.
                    # The normalization tail is decoupled: ya/yb are copied
                    # to SBUF right after the accumulation stops (freeing
                    # the PSUM tiles for the next chunk), and the broadcast
                    # matmul is deferred into the next chunk's block loop.
                    AVLAG = 2
                    for qc in range(QC if "noattn" not in probe else 0):
                        nkc = 4 * (qc + 1)
                        qs = slice(qc * 512, (qc + 1) * 512)
                        ya = yps.tile([65, 512], F32, tag="ya", name="ya")
                        yb = yps.tile([65, 512], F32, tag="yb", name="yb")
                        pend = []
                        for kc in range(nkc):
                            ks = slice(kc * 128, (kc + 1) * 128)
                            spair = mmps.tile([128, 1024], F32, tag="mm",
                                              name="spair")
                            nc.tensor.matmul(spair[:, 0:512], kt[0:64, ks],
                                             qt[0:64, qs], start=True, stop=True)
                            nc.tensor.matmul(spair[:, 512:1024], kt[64:128, ks],
                                             qt[64:128, qs], start=True, stop=True)
                            pp = pt_pool.tile([128, 1024], BF16, tag="pp")
                            nc.scalar.activation(pp, spair, Act.Exp, scale=0.125)
                            if kc >= 4 * qc and "nomask" not in probe:
                                nc.gpsimd.affine_select(
                                    out=pp, in_=pp,
                                    compare_op=mybir.AluOpType.is_ge,
                                    fill=0.0,
                                    base=qc * 512 - kc * 128,
                                    channel_multiplier=-1,
                                    pattern=[[0, 2], [1, 512]])
                            if "noexp" in probe:
                                pp = ppc
                            pend.append((kc, pp))
                            if kc == 0 and pending_bc:
                                pending_bc.pop()()

                            def _av(pkc, ppp, nkc=nkc, ya=ya, yb=yb, j=j):
                                nc.tensor.matmul(
                                    ya, vp_sb[:, pkc, 2 * j, :],
                                    ppp[:, 0:512],
                                    start=(pkc == 0), stop=(pkc == nkc - 1))
                                nc.tensor.matmul(
                                    yb, vp_sb[:, pkc, 2 * j + 1, :],
                                    ppp[:, 512:1024],
                                    start=(pkc == 0), stop=(pkc == nkc - 1))

                            if len(pend) > AVLAG:
                                _av(*pend.pop(0))
                        for pkc, ppp in pend:
                            _av(pkc, ppp)
                        pend = []
                        # tail: free ya/yb fast, normalize off the hot path
                        yua = yu_pool.tile([65, 512], F32, tag="yua")
                        yub = yu_pool.tile([65, 512], F32, tag="yub")
                        nc.vector.tensor_copy(yua, ya)
                        nc.vector.tensor_copy(yub, yb)
                        ra = ra_pool.tile([65, 512], mybir.dt.float32r,
                                          tag="ra")
                        rb = ra_pool.tile([65, 512], mybir.dt.float32r,
                                          tag="rb")
                        nc.vector.reciprocal(ra[64:65, 0:512], yua[64:65, :])
                        nc.vector.reciprocal(rb[64:65, 0:512], yub[64:65, :])

                        def _bc_mm(ra=ra, rb=rb, yua=yua, yub=yub, qs=qs, j=j,
                                   qc=qc):
                            bc = mmps.tile([128, 1024], F32, tag="mm",
                                           name="bc")
                            nc.tensor.matmul(bc[0:64, 0:512],
                                             ones_sb[64:65, :],
                                             ra[64:65, 0:512],
                                             start=True, stop=True)
                            nc.tensor.matmul(bc[0:64, 512:1024],
                                             ones_sb[64:65, :],
                                             rb[64:65, 0:512],
                                             start=True, stop=True)
                            nc.vector.tensor_mul(yt_sb[0:64, j, qs],
                                                 yua[0:64, :], bc[0:64, 0:512])
                            tb = tb_pool.tile([64, 512], BF16)
                            nc.vector.tensor_mul(tb, yub[0:64, :],
                                                 bc[0:64, 512:1024])
                            nc.sync.dma_start(yt_sb[64:128, j, qs], tb)
                            if debug:
                                nc.sync.dma_start(
                                    dbg_ra.ap()[j * QC + qc, 64:65],
                                    ra[64:65, :].bitcast(F32))

                        if "nonorm" in probe:
                            nc.vector.tensor_copy(yt_sb[0:64, j, qs],
                                                  yua[0:64, :])
                        else:
                            pending_bc.append(_bc_mm)

                    if debug:
                        nc.sync.dma_start(dbg_qk.ap()[j], qt[:])
                        nc.sync.dma_start(dbg_qk.ap()[NP + j], kt[:])

                for f in pending_bc:
                    f()
                pending_bc = []

                if debug:
                    nc.sync.dma_start(
                        dbg_vp.ap(),
                        vp_sb[:].rearrange("p a b c -> p (a b c)"))
                    for j in range(NP):
                        nc.sync.dma_start(dbg_yt.ap()[j], yt_sb[:, j, :])

            # ---- output projection ------------------------------------
            with tc.tile_pool(name="osb", bufs=2) as osb_pool, \
                 tc.tile_pool(name="pps", bufs=3, space="PSUM") as pps:
                ccw = 384
                if "noproj" in probe:
                    nc.sync.dma_start(out_d.ap()[0:128, :], yt_sb[:, 0, 0:C])
                for tg in range(TT // 4 if "noproj" not in probe else 0):
                    osb = osb_pool.tile([128, 4, C], BF16)
                    for a in range(4):
                        tt = tg * 4 + a
                        for c2 in range(C // ccw):
                            cs = slice(c2 * ccw, (c2 + 1) * ccw)
                            psum = pps.tile([128, ccw], F32)
                            for k in range(PC):
                                nc.tensor.matmul(
                                    psum, yt_sb[:, k, tt * 128:(tt + 1) * 128],
                                    wpv(k)[:, cs],
                                    start=(k == 0), stop=(k == PC - 1))
                            if c2 == 0:
                                nc.vector.tensor_copy(osb[:, a, cs], psum)
                            else:
                                nc.scalar.copy(osb[:, a, cs], psum)
                    nc.sync.dma_start(
                        out_d.ap()[tg * 512:(tg + 1) * 512, :]
                             .rearrange("(a p) c -> p a c", p=128),
                        osb[:])

    nc.compile()
    return nc


class _Runner:
    """Cached-jit SPMD runner (mirrors bass2jax.run_bass_via_pjrt, reusable)."""

    def __init__(self, nc, n_cores):
        import jax
        from jax.sharding import Mesh, PartitionSpec
        from jax.experimental.shard_map import shard_map
        import concourse.mybir as mybir
        from concourse import bass2jax

        bass2jax.install_neuronx_cc_hook()
        self.n_cores = n_cores
        part_name = (nc.partition_id_tensor.name
                     if nc.partition_id_tensor is not None else None)
        in_names, out_names, out_avals, zero_outs = [], [], [], []
        for alloc in nc.m.functions[0].allocations:
            if not isinstance(alloc, mybir.MemoryLocationSet):
                continue
            name = alloc.memorylocations[0].name
            if alloc.kind == "ExternalInput":
                if name != part_name:
                    in_names.append(name)
            elif alloc.kind == "ExternalOutput":
                out_names.append(name)
                shape = tuple(alloc.tensor_shape)
                dtype = mybir.dt.np(alloc.dtype)
                out_avals.append(jax.core.ShapedArray(shape, dtype))
                zero_outs.append(np.zeros(shape, dtype))
        self.in_names, self.out_names = in_names, out_names
        self.out_avals, self.zero_outs = out_avals, zero_outs
        all_names = in_names + out_names
        if part_name is not None:
            all_names = all_names + [part_name]

        def _body(*args):
            operands = list(args)
            if part_name is not None:
                operands.append(bass2jax.partition_id_tensor())
            return tuple(bass2jax._bass_exec_p.bind(
                *operands,
                out_avals=tuple(out_avals),
                in_names=tuple(all_names),
                out_names=tuple(out_names),
                lowering_input_output_aliases=(),
                sim_require_finite=True,
                sim_require_nnan=True,
                nc=nc,
            ))

        devices = jax.devices()[:n_cores]
        mesh = Mesh(np.asarray(devices), ("core",))
        nin = len(in_names) + len(out_names)
        self._fn = jax.jit(
            shard_map(_body, mesh=mesh,
                      in_specs=(PartitionSpec("core"),) * nin,
                      out_specs=(PartitionSpec("core"),) * len(out_names),
                      check_rep=False),
            keep_unused=True)

    def run(self, in_maps):
        args = [np.concatenate([np.asarray(m[name]) for m in in_maps], axis=0)
                for name in self.in_names]
        args += [np.zeros((self.n_cores * z.shape[0], *z.shape[1:]), z.dtype)
                 for z in self.zero_outs]
        outs = self._fn(*args)
        res = []
        for c in range(self.n_cores):
            d = {}
            for i, name in enumerate(self.out_names):
                per = np.asarray(outs[i]).reshape(
                    self.n_cores, *self.out_avals[i].shape)
                d[name] = per[c]
            res.append(d)
        return res


def _run(nc, in_maps):
    key = ("runner", id(nc))
    if key not in _CACHE:
        _CACHE[key] = _Runner(nc, len(in_maps))
    import types
    return types.SimpleNamespace(results=_CACHE[key].run(in_maps))


def build_in_maps(x, W, b, Wp, T, C):
    """Host-side packing of the per-core wall1/wall2 tensors (bf16)."""
    KT = C // 128
    cc, ss = _rope_tables(T)
    heads01 = np.arange(128) // 64
    jj = np.arange(128) % 64
    in_maps = []
    for c in range(NCORES):
        bb, g = divmod(c, 2)
        s = 384 * g
        wqk = np.empty((C, 768), np.float32)
        bqk = np.zeros((128, 16), np.float32)
        for m in range(3):
            for h01 in range(2):
                h = 2 * m + h01
                cols = slice(m * 128 + h01 * 64, m * 128 + h01 * 64 + 64)
                wqk[:, cols] = W[:, s + h * 64 + _PERM]
                wqk[:, 384 + m * 128 + h01 * 64:
                       384 + m * 128 + h01 * 64 + 64] = \
                    W[:, 768 + s + h * 64 + _PERM]
            bqk[:, m] = b[s + (2 * m + heads01) * 64 + _PERM[jj]]
            bqk[:, 3 + m] = b[768 + s + (2 * m + heads01) * 64 + _PERM[jj]]
        xt = np.ascontiguousarray(x[bb].T)                      # [C, T]
        wvh = W[:, 1536 + s:1536 + s + 384]                     # [C, 384]
        wph = Wp[s:s + 384, :]                                  # [384, C]
        wall1 = np.concatenate([
            xt.reshape(KT, 128, T).transpose(1, 0, 2).reshape(128, KT * T),
            wqk.reshape(KT, 128, 768).transpose(1, 0, 2).reshape(128, -1),
            cc, ss, bqk], axis=1).astype(bfloat16)
        wall2 = np.concatenate([
            wvh.reshape(KT, 128, 384).transpose(1, 0, 2).reshape(128, -1),
            wph.reshape(3, 128, C).transpose(1, 0, 2).reshape(128, -1),
        ], axis=1).astype(bfloat16)
        in_maps.append({"wall1": np.ascontiguousarray(wall1),
                        "wall2": np.ascontiguousarray(wall2)})
    return in_maps


def kernel(**inputs):
    x = np.ascontiguousarray(np.asarray(inputs["x"], dtype=np.float32))
    W = np.asarray(inputs["W_attn"], dtype=np.float32)
    b = np.asarray(inputs["b_attn"], dtype=np.float32)
    Wp = np.asarray(inputs["W_proj"], dtype=np.float32)
    bp = np.asarray(inputs["b_proj"], dtype=np.float32)
    B, T, C = x.shape

    if "nc" not in _CACHE:
        _CACHE["nc"] = build_nc(C, T, 6)
        _CACHE["build_args"] = (C, T, 6)
    nc = _CACHE["nc"]

    in_maps = build_in_maps(x, W, b, Wp, T, C)
    _CACHE["in_maps"] = in_maps
    res = _run(nc, in_maps).results
    extra = (bp + b[1536:2304] @ Wp).astype(np.float32)  # [C]
    out = np.empty((B, T, C), dtype=np.float32)
    for bb in range(B):
        out[bb] = (res[2 * bb]["out"].astype(np.float32)
                   + res[2 * bb + 1]["out"].astype(np.float32) + extra)
    return out
